# revision 1
# baseline (speedup 1.0000x reference)
"""Trainium2 Bass kernel for nn_AttentionAddition (8-core SPMD).

Sharding: data-parallel over the N (RoI) dimension. Each core owns Q = N/8
queries. K/V for the attention (kp = relu(sem) @ Wqk^T plus dummy row,
vv = comb) are computed shard-wise and exchanged with 2 pipelined AllGather
waves in bf16. Everything else is fp32.

Layout convention on device: activations are kept TRANSPOSED, i.e.
[feature, query] with the feature dim on SBUF partitions (tiles are
[128, n_chunks, q]). In this orientation every Linear of the module maps to
matmul(out, lhsT=W^T chunk, rhs=activation chunk) with a per-partition bias
via the ScalarE activation op, and no activation ever needs a transpose.
The single final transpose back to [query, feature] is done on the PE.
"""

import numpy as np
import ml_dtypes

import concourse.bass as bass
import concourse.tile as tile
from concourse import bacc, mybir
from concourse.masks import make_identity

F32 = mybir.dt.float32
BF16 = mybir.dt.bfloat16
AF = mybir.ActivationFunctionType
ALU = mybir.AluOpType

NCORES = 8
N, D, S, C = 8192, 1024, 300, 80
Q = N // NCORES          # queries per core = 1024
DC = D // 128            # feature chunks = 8
QC = Q // 128            # query chunks per core = 8
SCALE = 1.0 / np.sqrt(np.float32(D))  # 1/32


def _mk_ap(base_ap, offset_elems, dims):
    """Raw strided AP on a DRAM tensor. dims = [[step, count], ...]."""
    return bass.AP(
        tensor=base_ap.tensor,
        offset=base_ap.offset + offset_elems,
        ap=[list(d) for d in dims],
    )


class Ctx:
    pass


def build(debug=False, single=False, upto="abc", repeat=1):
    nc = bacc.Bacc("TRN2", target_bir_lowering=False, debug=False,
                   num_devices=1 if single else NCORES)
    cx = Ctx()
    cx.debug = debug
    cx.single = single
    cx.upto = upto

    def din(name, shape, dt=F32):
        return nc.dram_tensor(name, shape, dt, kind="ExternalInput").ap()

    cx.fpT = din("fpT", [D, Q], BF16)               # feature_pooled shard, transposed
    cx.ohT = din("ohT", [C + 1, Q], BF16)           # one-hot(gt_classes), transposed
    cx.cembT = din("cembT", [384, C + 1])     # [class_embed;bg].T pad, row 300 = 1
    cx.wprojT = din("wprojT", [384, D])       # w_proj.T pad, row 300 = b_proj
    cx.wcombT = din("wcombT", [2 * D + 1, D], BF16)  # w_comb.T, last row = b_comb
    cx.wqkT = din("wqkT", [D, D], BF16)
    cx.dumT = din("dumT", [D, 1], BF16)
    cx.w1T = din("w1T", [D, D // 2], BF16)
    cx.b1 = din("b1", [D // 2])
    cx.w2T = din("w2T", [D, D // 2], BF16)
    cx.b2 = din("b2", [D // 2])
    cx.w3T = din("w3T", [2 * D, D], BF16)
    cx.b3 = din("b3", [D])
    cx.wf1T = din("wf1T", [D, D], BF16)
    cx.bf1 = din("bf1", [D])
    cx.wf2T = din("wf2T", [D, D], BF16)
    cx.bf2 = din("bf2", [D])
    cx.lng = din("lng", [D])
    cx.lnb = din("lnb", [D])

    cx.out_d = nc.dram_tensor("out", [Q, D], F32, kind="ExternalOutput").ap()

    cx.dbg = {}
    if debug:
        def dout(name, shape, dt=F32):
            cx.dbg[name] = nc.dram_tensor(name, shape, dt,
                                          kind="ExternalOutput").ap()
        dout("d_ce", [C + 1, D], BF16)
        dout("d_semT", [128, DC * Q], BF16)
        dout("d_comb", [128, QC * D], BF16)
        dout("d_kpT", [128, DC * Q], BF16)
        dout("d_qpT", [128, DC * Q], BF16)
        dout("d_rowsum", [1, Q])
        dout("d_outacc", [128, DC * Q])
        dout("d_oT", [128, DC * Q], BF16)
        dout("d_normT", [128, DC * Q])

    # AllGather buffers, split so the kp gather can fire before comb is done.
    # Wave w = the w-th half of every rank's local keys. kp blocks are
    # kpT[:, w*512:+512] flattened ([1024 d, 512 k] row-major as [512, 1024]);
    # vv blocks are vv[w*512:+512, :] (natural [512 k, 1024 d]).
    cx.bounce_kp = [nc.dram_tensor(f"bkp{w}", [Q // 2, D], BF16,
                                   kind="Internal").ap() for w in range(2)]
    cx.bounce_vv = [nc.dram_tensor(f"bvv{w}", [Q // 2, D], BF16,
                                   kind="Internal").ap() for w in range(2)]
    cx.ag_kp = [nc.dram_tensor(f"agkp{w}", [NCORES * Q // 2, D], BF16,
                               kind="Internal", addr_space="Shared").ap()
                for w in range(2)]
    cx.ag_vv = [nc.dram_tensor(f"agvv{w}", [NCORES * Q // 2, D], BF16,
                               kind="Internal", addr_space="Shared").ap()
                for w in range(2)]

    with tile.TileContext(nc) as tc:
        with tc.tile_pool(name="pp", bufs=1) as pp:
            # all small fp32 constants packed into one 4KB-padded tile
            consts = pp.tile([128, 288], F32)
            cx.ident = consts[:, 0:128]
            make_identity(nc, cx.ident)
            cx.ones_c = consts[:, 128:129]        # ones col (partition reduce)
            nc.vector.memset(cx.ones_c, 1.0)
            cx.eps_t = consts[0:1, 129:130]
            nc.vector.memset(cx.eps_t, 1e-5)
            cx.ones_r = consts[0:1, 130:258]      # ones row (bias mm / bcast)
            nc.vector.memset(cx.ones_r, 1.0)
            cb = pp.tile([128, 130], BF16)
            cx.ones_cb = cb[:, 0:1]
            nc.vector.memset(cx.ones_cb, 1.0)
            cx.ones_rb = cb[0:1, 2:130]
            nc.vector.memset(cx.ones_rb, 1.0)

            for _rep in range(repeat):
                with tc.tile_pool(name="pq", bufs=1) as pq:
                    cx.pq = pq
                    _phase_a(nc, tc, cx)
                    if cx.upto == "a":
                        nc.gpsimd.dma_start(
                            out=cx.out_d.rearrange("(c p) d -> p c d", p=128),
                            in_=cx.qpT[:])
                    else:
                        with tc.tile_pool(name="pbc", bufs=1) as pbc:
                            cx.pbc = pbc
                            _phase_b(nc, tc, cx)
                            if cx.upto == "ab":
                                nc.sync.dma_start(
                                    out=cx.out_d
                                    .rearrange("(c p) d -> p c d", p=128),
                                    in_=cx.outn[:])
                            else:
                                _phase_c(nc, tc, cx)
    nc.compile()
    return nc


def _phase_a(nc, tc, cx):
    """Projections. Order chosen so attention unblocks ASAP: ce -> visT ->
    qp -> semr -> kp (+kp AllGather per wave) -> semT -> comb (+vv AllGather).
    """
    debug, dbg = cx.debug, cx.dbg

    def ship(bounce, agb, bounce_ap, in_ap):
        nc.sync.dma_start(out=bounce_ap, in_=in_ap)
        if cx.single:
            for r in range(NCORES):
                nc.sync.dma_start(
                    out=agb[r * (Q // 2):(r + 1) * (Q // 2), :], in_=bounce)
        else:
            nc.gpsimd.collective_compute(
                "AllGather", ALU.bypass,
                replica_groups=[list(range(NCORES))],
                ins=[bounce], outs=[agb])

    with (
        tc.tile_pool(name="paV", bufs=1) as paV,
        tc.tile_pool(name="pap", bufs=5, space="PSUM") as pap,
    ):
        oh_sb = paV.tile([C + 1, Q], BF16)
        nc.sync.dma_start(out=oh_sb[:], in_=cx.ohT)
        ceR_sb = paV.tile([C + 1, D], BF16)

        with tc.tile_pool(name="pa0", bufs=1) as pa0:
            fpT_sb = pa0.tile([128, DC, Q], BF16)
            nc.sync.dma_start(out=fpT_sb[:],
                              in_=cx.fpT.rearrange("(c p) q -> p c q", p=128))

            # ce = [cemb;bg;1] @ [w_proj.T;b_proj]  -> [81, D]
            cembT_sb = pa0.tile([128, 3, C + 1], F32)
            nc.sync.dma_start(out=cembT_sb[:],
                              in_=cx.cembT.rearrange("(c p) n -> p c n", p=128))
            wprojT_sb = pa0.tile([128, 3, D], F32)
            nc.sync.dma_start(out=wprojT_sb[:],
                              in_=cx.wprojT.rearrange("(c p) d -> p c d", p=128))
            ce_ps = pap.tile([C + 1, D], F32, tag="ps2", bufs=1)
            for ob in range(2):
                for sc in range(3):
                    nc.tensor.matmul(ce_ps[:, ob * 512:(ob + 1) * 512],
                                     cembT_sb[:, sc, :],
                                     wprojT_sb[:, sc, ob * 512:(ob + 1) * 512],
                                     start=(sc == 0), stop=(sc == 2))
            ce_sb = pa0.tile([C + 1, D], BF16)
            nc.scalar.copy(out=ce_sb[:], in_=ce_ps[:])
            nc.scalar.activation(out=ceR_sb[:], in_=ce_ps[:], func=AF.Relu)
            if debug:
                nc.sync.dma_start(out=dbg["d_ce"], in_=ce_sb[:])

            # visT = relu(fpT); lives until phase C (z1/z2 and the o3 concat)
            cx.visT = visT = cx.pq.tile([128, DC, Q], BF16, name="visT")
            for dc in range(DC):
                nc.scalar.activation(out=visT[:, dc, :], in_=fpT_sb[:, dc, :],
                                     func=AF.Relu)

            with tc.tile_pool(name="paK", bufs=1) as paK:
                # qp = wqk @ visT (queries unblock attention S matmuls)
                wqk_sb = paK.tile([128, DC, D], BF16)
                nc.sync.dma_start(out=wqk_sb[:],
                                  in_=cx.wqkT.rearrange("(c p) o -> p c o", p=128))
                cx.qpT = cx.pq.tile([128, DC, Q], BF16, name="qpT")
                for w in range(2):
                    qs = slice(w * 512, (w + 1) * 512)
                    for oc in range(DC):
                        qk_ps = pap.tile([128, 512], F32, tag="ps")
                        for ic in range(DC):
                            nc.tensor.matmul(qk_ps[:],
                                             wqk_sb[:, ic, oc * 128:(oc + 1) * 128],
                                             visT[:, ic, qs],
                                             start=(ic == 0), stop=(ic == DC - 1))
                        nc.scalar.copy(out=cx.qpT[:, oc, qs], in_=qk_ps[:])
                if debug:
                    nc.sync.dma_start(out=dbg["d_qpT"],
                                      in_=cx.qpT[:].rearrange("p c q -> p (c q)"))

                # semrT = gather of relu(ce); kp = wqk @ semrT, shipped per wave
                semrT = paK.tile([128, DC, Q], BF16)
                for dc in range(DC):
                    for qh in range(Q // 512):
                        qs = slice(qh * 512, (qh + 1) * 512)
                        sem_ps = pap.tile([128, 512], F32, tag="ps")
                        nc.tensor.matmul(sem_ps[:],
                                         ceR_sb[:, dc * 128:(dc + 1) * 128],
                                         oh_sb[:, qs], start=True, stop=True)
                        nc.scalar.copy(out=semrT[:, dc, qs], in_=sem_ps[:])

                kp_sb = paK.tile([128, DC, Q], BF16)
                for w in range(2):
                    qs = slice(w * 512, (w + 1) * 512)
                    for oc in range(DC):
                        qk_ps = pap.tile([128, 512], F32, tag="ps")
                        for ic in range(DC):
                            nc.tensor.matmul(qk_ps[:],
                                             wqk_sb[:, ic, oc * 128:(oc + 1) * 128],
                                             semrT[:, ic, qs],
                                             start=(ic == 0), stop=(ic == DC - 1))
                        nc.scalar.copy(out=kp_sb[:, oc, qs], in_=qk_ps[:])
                    ship(cx.bounce_kp[w], cx.ag_kp[w],
                         _mk_ap(cx.bounce_kp[w], 0,
                                [[512, 128], [65536, 8], [1, 512]]),
                         kp_sb[:, :, qs])
                if debug:
                    nc.sync.dma_start(out=dbg["d_kpT"],
                                      in_=kp_sb[:].rearrange("p c q -> p (c q)"))

            # semT gather then comb (natural [q, d]), shipped per wave
            semT = pa0.tile([128, DC, Q], BF16)
            for dc in range(DC):
                for qh in range(Q // 512):
                    qs = slice(qh * 512, (qh + 1) * 512)
                    sem_ps = pap.tile([128, 512], F32, tag="ps")
                    nc.tensor.matmul(sem_ps[:],
                                     ce_sb[:, dc * 128:(dc + 1) * 128],
                                     oh_sb[:, qs], start=True, stop=True)
                    nc.scalar.copy(out=semT[:, dc, qs], in_=sem_ps[:])
            if debug:
                nc.sync.dma_start(out=dbg["d_semT"],
                                  in_=semT[:].rearrange("p c q -> p (c q)"))

            comb_sb = paV.tile([128, QC, D], BF16, name="comb_sb")
            wcb_row = pa0.tile([1, D], BF16)
            nc.sync.dma_start(out=wcb_row[:], in_=cx.wcombT[2 * D:2 * D + 1, :])
            with tc.tile_pool(name="paw", bufs=2) as paw:
                for w in range(2):
                    for ob in range(2):
                        os_ = slice(ob * 512, (ob + 1) * 512)
                        wcq = paw.tile([128, 16, 512], BF16, tag="wcq")
                        nc.sync.dma_start(
                            out=wcq[:],
                            in_=cx.wcombT[0:2 * D, os_]
                            .rearrange("(c p) o -> p c o", p=128))
                        for qc in range(w * 4, w * 4 + 4):
                            cb_ps = pap.tile([128, 512], F32, tag="ps")
                            for ic in range(16):
                                lhs = (semT[:, ic, qc * 128:(qc + 1) * 128]
                                       if ic < 8 else
                                       fpT_sb[:, ic - 8, qc * 128:(qc + 1) * 128])
                                nc.tensor.matmul(cb_ps[:], lhs, wcq[:, ic, :],
                                                 start=(ic == 0), stop=False)
                            nc.tensor.matmul(cb_ps[:], cx.ones_rb,
                                             wcb_row[:, os_],
                                             start=False, stop=True)
                            nc.scalar.copy(out=comb_sb[:, qc, os_], in_=cb_ps[:])
                    ship(cx.bounce_vv[w], cx.ag_vv[w],
                         _mk_ap(cx.bounce_vv[w], 0,
                                [[1024, 128], [131072, 4], [1, 1024]]),
                         comb_sb[:, w * 4:(w + 1) * 4, :])
            if debug:
                nc.sync.dma_start(out=dbg["d_comb"],
                                  in_=comb_sb[:].rearrange("p c q -> p (c q)"))


def _phase_b(nc, tc, cx):
    """Attention: S^T = kp^T-chunks x qpT, E = exp(S/32), out^T += vv^T E.

    N=512 structure: per 2048-key superblock j, first compute all 32 E tiles
    (16 key-chunks x 2 query-halves, free dim 512), then do the PV matmuls in
    two d-half passes per query-half so the PV accumulator fits in 4 PSUM
    banks. PSUM: S 2 + rowsum 2 + PV 4 = 8 banks.
    """
    debug, dbg = cx.debug, cx.dbg
    qpT = cx.qpT
    cx.rowsum = rowsum = cx.pbc.tile([1, Q], F32, name="rowsum")
    out_acc = cx.pbc.tile([128, DC, Q], F32, name="out_acc")
    with (
        tc.tile_pool(name="pb", bufs=1) as pb,
        tc.tile_pool(name="pkv", bufs=5) as pkv,
        tc.tile_pool(name="pe", bufs=33) as pe,
        tc.tile_pool(name="pbo", bufs=1, space="PSUM") as pbo,
        tc.tile_pool(name="pbs", bufs=2, space="PSUM") as pbs,
        tc.tile_pool(name="pbr", bufs=2, space="PSUM") as pbr,
    ):
        # dummy-key contribution to the softmax denominator
        dum_sb = pb.tile([128, DC, 1], BF16)
        nc.sync.dma_start(out=dum_sb[:],
                          in_=cx.dumT.rearrange("(c p) o -> p c o", p=128))
        for qh in range(Q // 512):
            qs = slice(qh * 512, (qh + 1) * 512)
            sd_ps = pbs.tile([1, 512], F32, tag="sps")
            for dc in range(DC):
                nc.tensor.matmul(sd_ps[:], dum_sb[:, dc, :], qpT[:, dc, qs],
                                 start=(dc == 0), stop=(dc == DC - 1))
            nc.scalar.activation(out=rowsum[:, qs], in_=sd_ps[:],
                                 func=AF.Exp, scale=float(SCALE))

        for j in range(4):                # key superblocks of 2048 keys
            w, rr = j // 2, (j % 2) * 4
            kp_t, vv_t = [], []
            for s in range(4):
                r = rr + s
                kt = pkv.tile([128, DC, 512], BF16, tag="kp")
                nc.sync.dma_start(
                    out=kt[:],
                    in_=_mk_ap(cx.ag_kp[w], r * (Q // 2) * D,
                               [[512, 128], [65536, 8], [1, 512]]))
                vt = pkv.tile([128, 4, D], BF16, tag="vv")
                nc.sync.dma_start(
                    out=vt[:],
                    in_=cx.ag_vv[w][r * (Q // 2):(r + 1) * (Q // 2), :]
                    .rearrange("(kc p) d -> p kc d", p=128))
                kp_t.append(kt)
                vv_t.append(vt)

            # S + exp for all 16 key chunks x 2 query halves; accumulate rowsum
            e_t = [[None, None] for _ in range(16)]
            r_ps = [pbr.tile([1, 512], F32, tag="rps", name=f"rps{_qh}")
                    for _qh in range(2)]
            for sk in range(16):
                s, kc = sk // 4, sk % 4
                for qh in range(2):
                    qs = slice(qh * 512, (qh + 1) * 512)
                    s_ps = pbs.tile([128, 512], F32, tag="sps")
                    for dc in range(DC):
                        nc.tensor.matmul(
                            s_ps[:],
                            kp_t[s][:, dc, kc * 128:(kc + 1) * 128],
                            qpT[:, dc, qs],
                            start=(dc == 0), stop=(dc == DC - 1))
                    et = pe.tile([128, 512], BF16, tag="et")
                    nc.scalar.activation(out=et[:], in_=s_ps[:],
                                         func=AF.Exp, scale=float(SCALE))
                    e_t[sk][qh] = et
                    nc.tensor.matmul(r_ps[qh][:], cx.ones_cb, et[:],
                                     start=(sk == 0), stop=(sk == 15))
            for qh in range(2):
                qs = slice(qh * 512, (qh + 1) * 512)
                nc.vector.tensor_add(rowsum[:, qs], rowsum[:, qs], r_ps[qh][:])

            # PV: two d-half passes per query half (4 PSUM banks each)
            for qh in range(2):
                qs = slice(qh * 512, (qh + 1) * 512)
                for dh in range(2):
                    o_ps = pbo.tile([128, 4, 512], F32, tag="ops")
                    for sk in range(16):
                        s, kc = sk // 4, sk % 4
                        for d4 in range(4):
                            dc = dh * 4 + d4
                            nc.tensor.matmul(
                                o_ps[:, d4, :],
                                vv_t[s][:, kc, dc * 128:(dc + 1) * 128],
                                e_t[sk][qh][:],
                                start=(sk == 0), stop=(sk == 15))
                    for d4 in range(4):
                        dc = dh * 4 + d4
                        if j == 0:
                            nc.vector.tensor_copy(out_acc[:, dc, qs],
                                                  o_ps[:, d4, :])
                        else:
                            nc.vector.tensor_add(out_acc[:, dc, qs],
                                                 out_acc[:, dc, qs],
                                                 o_ps[:, d4, :])

        if debug:
            nc.sync.dma_start(out=dbg["d_rowsum"], in_=rowsum[:])
            nc.sync.dma_start(out=dbg["d_outacc"],
                              in_=out_acc[:].rearrange("p c q -> p (c q)"))

        # normalize in place: out_acc /= rowsum (broadcast along partitions)
        recip = pb.tile([1, Q], F32)
        nc.vector.reciprocal(recip[:], rowsum[:])
        recipb = pb.tile([128, Q], F32)
        for qh in range(Q // 512):
            qs = slice(qh * 512, (qh + 1) * 512)
            b_ps = pbs.tile([128, 512], F32, tag="sps")
            nc.tensor.matmul(b_ps[:], cx.ones_r, recip[:, qs],
                             start=True, stop=True)
            nc.scalar.copy(out=recipb[:, qs], in_=b_ps[:])
        for dc in range(DC):
            nc.vector.tensor_mul(out_acc[:, dc, :], out_acc[:, dc, :], recipb[:])
        cx.outn = out_acc


def _phase_c(nc, tc, cx):
    """Epilogue: o1/o2/o3, LayerNorm, FFN, final relu-add, transpose, store."""
    debug, dbg = cx.debug, cx.dbg
    outn = cx.outn
    with (
        tc.tile_pool(name="pcB", bufs=1) as pcB,
        tc.tile_pool(name="pcp", bufs=8, space="PSUM") as pcp,
    ):
        # all per-feature bias vectors packed into one 4KB tile
        bias = pcB.tile([128, 48], F32)
        b1_sb = bias[:, 0:4]
        nc.sync.dma_start(out=b1_sb, in_=cx.b1.rearrange("(c p) -> p c", p=128))
        b2_sb = bias[:, 4:8]
        nc.sync.dma_start(out=b2_sb, in_=cx.b2.rearrange("(c p) -> p c", p=128))
        b3_sb = bias[:, 8:16]
        nc.sync.dma_start(out=b3_sb, in_=cx.b3.rearrange("(c p) -> p c", p=128))
        bf1_sb = bias[:, 16:24]
        nc.sync.dma_start(out=bf1_sb, in_=cx.bf1.rearrange("(c p) -> p c", p=128))
        bf2_sb = bias[:, 24:32]
        nc.sync.dma_start(out=bf2_sb, in_=cx.bf2.rearrange("(c p) -> p c", p=128))
        lnb2_sb = bias[:, 32:40]                 # ln_b + bf2 folded
        nc.sync.dma_start(out=lnb2_sb, in_=cx.lnb.rearrange("(c p) -> p c", p=128))
        nc.vector.tensor_add(lnb2_sb, lnb2_sb, bf2_sb)
        lng_sb = bias[:, 40:48]
        nc.sync.dma_start(out=lng_sb, in_=cx.lng.rearrange("(c p) -> p c", p=128))

        with tc.tile_pool(name="pcOT", bufs=1) as pcOT:
            oT_sb = pcOT.tile([128, DC, Q], BF16)
            oT32 = pcOT.tile([128, DC, Q], F32)   # fp32 copy for the LN path
            cx._oT32 = oT32

            with tc.tile_pool(name="pcA", bufs=1) as pcA:
                vis2 = cx.visT
                o1_sb = pcA.tile([128, 4, Q], BF16)
                o2_sb = pcA.tile([128, 4, Q], BF16)
                with tc.tile_pool(name="pcZ", bufs=1) as pcZ:
                    w1_sb = pcZ.tile([128, DC, 512], BF16)
                    nc.sync.dma_start(out=w1_sb[:],
                                      in_=cx.w1T.rearrange("(c p) o -> p c o", p=128))
                    w2_sb = pcZ.tile([128, DC, 512], BF16)
                    nc.sync.dma_start(out=w2_sb[:],
                                      in_=cx.w2T.rearrange("(c p) o -> p c o", p=128))
                    for half, (o_sb, wh_sb, bh_sb) in enumerate(
                            [(o1_sb, w1_sb, b1_sb), (o2_sb, w2_sb, b2_sb)]):
                        for qh in range(Q // 512):
                            qs = slice(qh * 512, (qh + 1) * 512)
                            z_sb = pcZ.tile([128, DC, 512], BF16, tag="z", bufs=1)
                            for dc in range(DC):
                                if half == 0:
                                    nc.vector.tensor_mul(z_sb[:, dc, :],
                                                         outn[:, dc, qs],
                                                         vis2[:, dc, qs])
                                else:
                                    nc.vector.tensor_sub(z_sb[:, dc, :],
                                                         vis2[:, dc, qs],
                                                         outn[:, dc, qs])
                            for oc in range(4):
                                m_ps = pcp.tile([128, 512], F32, tag="cps")
                                for ic in range(DC):
                                    nc.tensor.matmul(
                                        m_ps[:],
                                        wh_sb[:, ic, oc * 128:(oc + 1) * 128],
                                        z_sb[:, ic, :],
                                        start=(ic == 0), stop=(ic == DC - 1))
                                nc.scalar.activation(out=o_sb[:, oc, qs],
                                                     in_=m_ps[:], func=AF.Relu,
                                                     bias=bh_sb[:, oc:oc + 1])

                # o = w3 @ [o1; o2; vis] + b3  (transposed out [d, q])
                with tc.tile_pool(name="pcW", bufs=2) as pcW:
                    for oc in range(DC):
                        w3c = pcW.tile([128, 16, 128], BF16, tag="w3c")
                        nc.sync.dma_start(
                            out=w3c[:],
                            in_=cx.w3T[:, oc * 128:(oc + 1) * 128]
                            .rearrange("(c p) o -> p c o", p=128))
                        for qh in range(Q // 512):
                            qs = slice(qh * 512, (qh + 1) * 512)
                            m_ps = pcp.tile([128, 512], F32, tag="cps")
                            for ic in range(16):
                                rhs = (o1_sb[:, ic, qs] if ic < 4 else
                                       o2_sb[:, ic - 4, qs] if ic < 8 else
                                       vis2[:, ic - 8, qs])
                                nc.tensor.matmul(m_ps[:], w3c[:, ic, :], rhs,
                                                 start=(ic == 0), stop=(ic == 15))
                            nc.scalar.activation(out=oT_sb[:, oc, qs], in_=m_ps[:],
                                                 func=AF.Identity,
                                                 bias=b3_sb[:, oc:oc + 1])
                            nc.scalar.activation(out=oT32[:, oc, qs], in_=m_ps[:],
                                                 func=AF.Identity,
                                                 bias=b3_sb[:, oc:oc + 1])
            if debug:
                nc.sync.dma_start(out=dbg["d_oT"],
                                  in_=oT_sb[:].rearrange("p c q -> p (c q)"))

            with tc.tile_pool(name="pcN", bufs=1) as pcN:
                # LayerNorm over feature dim (partition reduce via ones-matmul)
                normT = pcN.tile([128, DC, Q], F32)
                with tc.tile_pool(name="pcL", bufs=2) as pcL:
                    for qh in range(Q // 512):
                        qs = slice(qh * 512, (qh + 1) * 512)
                        sum_ps = pcp.tile([1, 512], F32, tag="cps")
                        ssq_ps = pcp.tile([1, 512], F32, tag="cps")
                        for dc in range(DC):
                            nc.tensor.matmul(sum_ps[:], cx.ones_cb,
                                             oT_sb[:, dc, qs],
                                             start=(dc == 0), stop=(dc == DC - 1))
                            sq_t = pcL.tile([128, 512], BF16, tag="sq")
                            nc.scalar.activation(out=sq_t[:], in_=oT_sb[:, dc, qs],
                                                 func=AF.Square)
                            nc.tensor.matmul(ssq_ps[:], cx.ones_cb, sq_t[:],
                                             start=(dc == 0), stop=(dc == DC - 1))
                        st = pcL.tile([1, 3, 512], F32, tag="st", bufs=1)
                        slot_a, slot_b, slot_c = (st[:, i, :] for i in range(3))
                        nc.scalar.mul(out=slot_a, in_=sum_ps[:], mul=1.0 / D)  # mu
                        nc.scalar.mul(out=slot_b, in_=ssq_ps[:], mul=1.0 / D)  # E[x^2]
                        nc.vector.tensor_mul(slot_c, slot_a, slot_a)    # mu^2
                        nc.vector.tensor_sub(slot_b, slot_b, slot_c)    # var
                        nc.scalar.activation(out=slot_b, in_=slot_b, func=AF.Sqrt,
                                             bias=cx.eps_t)             # sd
                        nc.vector.reciprocal(slot_c, slot_b)            # c1 = rstd
                        nc.vector.tensor_mul(slot_a, slot_a, slot_c)    # c0 = mu*rstd
                        c1b = pcL.tile([128, 512], F32, tag="c1b")
                        c0b = pcL.tile([128, 512], F32, tag="c0b")
                        for src, dst in [(slot_c, c1b), (slot_a, c0b)]:
                            bb_ps = pcp.tile([128, 512], F32, tag="cps")
                            nc.tensor.matmul(bb_ps[:], cx.ones_r, src,
                                             start=True, stop=True)
                            nc.scalar.copy(out=dst[:], in_=bb_ps[:])
                        for dc in range(DC):
                            tmp = pcL.tile([128, 512], F32, tag="lnt")
                            nc.vector.tensor_mul(tmp[:], oT32[:, dc, qs], c1b[:])
                            nc.vector.tensor_sub(tmp[:], tmp[:], c0b[:])
                            nc.vector.tensor_scalar(
                                out=normT[:, dc, qs], in0=tmp[:],
                                scalar1=lng_sb[:, dc:dc + 1],
                                scalar2=lnb2_sb[:, dc:dc + 1],
                                op0=ALU.mult, op1=ALU.add)
                if debug:
                    nc.sync.dma_start(out=dbg["d_normT"],
                                      in_=normT[:].rearrange("p c q -> p (c q)"))

                # FFN layer 1 (consumes oT), weights streamed per output chunk
                f1_sb = pcN.tile([128, DC, Q], BF16)
                with tc.tile_pool(name="pcM1", bufs=2) as pcM1:
                    for oc in range(DC):
                        wf1c = pcM1.tile([128, DC, 128], BF16, tag="wf1c")
                        nc.sync.dma_start(
                            out=wf1c[:],
                            in_=cx.wf1T[:, oc * 128:(oc + 1) * 128]
                            .rearrange("(c p) o -> p c o", p=128))
                        for qh in range(Q // 512):
                            qs = slice(qh * 512, (qh + 1) * 512)
                            m_ps = pcp.tile([128, 512], F32, tag="cps")
                            for ic in range(DC):
                                nc.tensor.matmul(m_ps[:], wf1c[:, ic, :],
                                                 oT_sb[:, ic, qs],
                                                 start=(ic == 0),
                                                 stop=(ic == DC - 1))
                            nc.scalar.activation(out=f1_sb[:, oc, qs], in_=m_ps[:],
                                                 func=AF.Relu,
                                                 bias=bf1_sb[:, oc:oc + 1])

                # FFN layer 2 + LayerNorm residual + final relu
                with tc.tile_pool(name="pcM2", bufs=1) as pcM2:
                    fin_sb = pcM2.tile([128, DC, Q], F32)
                    with tc.tile_pool(name="pcM2w", bufs=2) as pcM2w:
                        for oc in range(DC):
                            wf2c = pcM2w.tile([128, DC, 128], BF16, tag="wf2c")
                            nc.sync.dma_start(
                                out=wf2c[:],
                                in_=cx.wf2T[:, oc * 128:(oc + 1) * 128]
                                .rearrange("(c p) o -> p c o", p=128))
                            for qh in range(Q // 512):
                                qs = slice(qh * 512, (qh + 1) * 512)
                                m_ps = pcp.tile([128, 512], F32, tag="cps")
                                for ic in range(DC):
                                    nc.tensor.matmul(m_ps[:], wf2c[:, ic, :],
                                                     f1_sb[:, ic, qs],
                                                     start=(ic == 0),
                                                     stop=(ic == DC - 1))
                                ts = pcM2w.tile([128, 512], F32, tag="ts")
                                nc.vector.tensor_add(ts[:], m_ps[:],
                                                     normT[:, oc, qs])
                                nc.scalar.activation(out=fin_sb[:, oc, qs],
                                                     in_=ts[:], func=AF.Relu)

                    # transpose [d, q] -> [q, d] on the PE
                    # (oT32 is dead after the LN stage; reuse its space)
                    onat = cx._oT32
                    for dc in range(DC):
                        for qc in range(QC):
                            t_ps = pcp.tile([128, 128], F32, tag="cps")
                            nc.tensor.transpose(
                                t_ps[:], fin_sb[:, dc, qc * 128:(qc + 1) * 128],
                                cx.ident)
                            nc.scalar.copy(
                                out=onat[:, qc, dc * 128:(dc + 1) * 128],
                                in_=t_ps[:])
                    nc.sync.dma_start(
                        out=cx.out_d.rearrange("(c p) d -> p c d", p=128),
                        in_=onat[:])


# ---------------------------------------------------------------------------
# Host side
# ---------------------------------------------------------------------------

_CACHE = {}


def _prep_in_maps(inputs):
    f32 = np.float32
    fp = np.asarray(inputs["feature_pooled"], f32)
    gt = np.asarray(inputs["gt_classes"]).astype(np.int64)
    ce = np.asarray(inputs["class_embed"], f32)
    bg = np.asarray(inputs["bg_embed"], f32)
    w_proj = np.asarray(inputs["w_proj"], f32)
    b_proj = np.asarray(inputs["b_proj"], f32)
    w_comb = np.asarray(inputs["w_comb"], f32)
    b_comb = np.asarray(inputs["b_comb"], f32)
    w_qk = np.asarray(inputs["w_qk"], f32)
    dummy = np.asarray(inputs["dummy"], f32)

    cembT = np.zeros((384, C + 1), f32)
    cembT[:S] = np.concatenate([ce, bg], 0).T
    cembT[S] = 1.0
    wprojT = np.zeros((384, D), f32)
    wprojT[:S] = w_proj.T
    wprojT[S] = b_proj
    wcombT = np.concatenate([w_comb.T, b_comb[None, :]], 0)

    shared = {
        "cembT": cembT,
        "wprojT": wprojT,
        "wcombT": np.ascontiguousarray(wcombT).astype(ml_dtypes.bfloat16),
        "wqkT": np.ascontiguousarray(w_qk.T).astype(ml_dtypes.bfloat16),
        "dumT": np.ascontiguousarray(dummy.T).astype(ml_dtypes.bfloat16),
        "w1T": np.ascontiguousarray(np.asarray(inputs["w1"], f32).T).astype(ml_dtypes.bfloat16),
        "b1": np.asarray(inputs["b1"], f32),
        "w2T": np.ascontiguousarray(np.asarray(inputs["w2"], f32).T).astype(ml_dtypes.bfloat16),
        "b2": np.asarray(inputs["b2"], f32),
        "w3T": np.ascontiguousarray(np.asarray(inputs["w3"], f32).T).astype(ml_dtypes.bfloat16),
        "b3": np.asarray(inputs["b3"], f32),
        "wf1T": np.ascontiguousarray(np.asarray(inputs["wf1"], f32).T).astype(ml_dtypes.bfloat16),
        "bf1": np.asarray(inputs["bf1"], f32),
        "wf2T": np.ascontiguousarray(np.asarray(inputs["wf2"], f32).T).astype(ml_dtypes.bfloat16),
        "bf2": np.asarray(inputs["bf2"], f32),
        "lng": np.asarray(inputs["ln_g"], f32),
        "lnb": np.asarray(inputs["ln_b"], f32),
    }
    in_maps = []
    for c in range(NCORES):
        qs = slice(c * Q, (c + 1) * Q)
        oh = np.zeros((C + 1, Q), ml_dtypes.bfloat16)
        oh[gt[qs], np.arange(Q)] = 1.0
        m = dict(shared)
        m["fpT"] = np.ascontiguousarray(fp[qs].T).astype(ml_dtypes.bfloat16)
        m["ohT"] = oh
        in_maps.append(m)
    return in_maps


def get_nc(debug=False):
    key = ("nc", debug)
    if key not in _CACHE:
        _CACHE[key] = build(debug=debug)
    return _CACHE[key]


def kernel(**inputs):
    from concourse import bass_utils
    try:
        # persistent XLA/PJRT compile cache so repeat invocations (fresh
        # processes included) skip the NEFF compile
        import jax
        jax.config.update("jax_compilation_cache_dir", "/tmp/jax_neff_cache")
        jax.config.update("jax_persistent_cache_min_compile_time_secs", 1.0)
        jax.config.update("jax_persistent_cache_min_entry_size_bytes", 0)
    except Exception:
        pass
    nc = get_nc(debug=False)
    in_maps = _prep_in_maps(inputs)
    res = bass_utils.run_bass_kernel_spmd(
        nc, in_maps, core_ids=list(range(NCORES)), trace=False)
    return np.concatenate([res.results[c]["out"] for c in range(NCORES)], axis=0)



# revision 3
# speedup vs baseline: 1.0461x; 1.0461x over previous
"""Trainium2 Bass kernel for nn_AttentionAddition (8-core SPMD).

Sharding: data-parallel over the N (RoI) dimension. Each core owns Q = N/8
queries. K/V for the attention (kp = relu(sem) @ Wqk^T plus dummy row,
vv = comb) are computed shard-wise and exchanged with 2 pipelined AllGather
waves in fp8 (e4m3).

The entire attention path (qp/kp projections, comb, S = qp kp^T, exp,
PV) runs in fp8 with DoubleRow matmuls (2 k-chunks of 128 contracted per
instruction). The epilogue (z1/z2, o3, LayerNorm, FFN) stays bf16 - fp8
there breaches the accuracy budget.

Layout convention on device: activations are kept TRANSPOSED, i.e.
[feature, query] with the feature dim on SBUF partitions (tiles are
[128, n_chunks, q]). In this orientation every Linear of the module maps to
matmul(out, lhsT=W^T chunk, rhs=activation chunk) with a per-partition bias
via the ScalarE activation op, and no activation ever needs a transpose.
The single final transpose back to [query, feature] is done on the PE.
"""

import numpy as np
import ml_dtypes

import concourse.bass as bass
import concourse.tile as tile
from concourse import bacc, mybir
from concourse.masks import make_identity

F32 = mybir.dt.float32
BF16 = mybir.dt.bfloat16
FP8 = mybir.dt.float8e4
AF = mybir.ActivationFunctionType
ALU = mybir.AluOpType
DR = mybir.MatmulPerfMode.DoubleRow

NCORES = 8
N, D, S, C = 8192, 1024, 300, 80
Q = N // NCORES          # queries per core = 1024
DC = D // 128            # feature chunks = 8
QC = Q // 128            # query chunks per core = 8
SCALE = 1.0 / np.sqrt(np.float32(D))  # 1/32


def _mk_ap(base_ap, offset_elems, dims):
    """Raw strided AP on a DRAM tensor. dims = [[step, count], ...]."""
    return bass.AP(
        tensor=base_ap.tensor,
        offset=base_ap.offset + offset_elems,
        ap=[list(d) for d in dims],
    )


class Ctx:
    pass


def build(debug=False, single=False, upto="abc", repeat=1):
    nc = bacc.Bacc("TRN2", target_bir_lowering=False, debug=False,
                   num_devices=1 if single else NCORES)
    cx = Ctx()
    cx.debug = debug
    cx.single = single
    cx.upto = upto

    def din(name, shape, dt=F32):
        return nc.dram_tensor(name, shape, dt, kind="ExternalInput").ap()

    cx.fpT = din("fpT", [D, Q], BF16)               # feature_pooled shard, transposed
    cx.fpT8 = din("fpT8", [D, Q], FP8)              # same, fp8 (comb lhsT)
    cx.ohT = din("ohT", [C + 1, Q], BF16)           # one-hot(gt_classes), transposed
    cx.cembT = din("cembT", [384, C + 1])     # [class_embed;bg].T pad, row 300 = 1
    cx.wprojT = din("wprojT", [384, D])       # w_proj.T pad, row 300 = b_proj
    cx.wcombT8 = din("wcombT8", [2 * D, D], FP8)    # w_comb.T fp8
    cx.wcb = din("wcb", [1, D])                     # b_comb
    cx.wqkT8 = din("wqkT8", [D, D], FP8)
    cx.dumT16 = din("dumT16", [D, 16], FP8)         # dummy.T in col 0, rest 0
    cx.w1T = din("w1T", [D, D // 2], BF16)
    cx.b1 = din("b1", [D // 2])
    cx.w2T = din("w2T", [D, D // 2], BF16)
    cx.b2 = din("b2", [D // 2])
    cx.w3T = din("w3T", [2 * D, D], BF16)
    cx.b3 = din("b3", [D])
    cx.wf1T = din("wf1T", [D, D], BF16)
    cx.bf1 = din("bf1", [D])
    cx.wf2T = din("wf2T", [D, D], BF16)
    cx.bf2 = din("bf2", [D])
    cx.lng = din("lng", [D])
    cx.lnb = din("lnb", [D])

    cx.out_d = nc.dram_tensor("out", [Q, D], F32, kind="ExternalOutput").ap()

    cx.dbg = {}
    if debug:
        def dout(name, shape, dt=F32):
            cx.dbg[name] = nc.dram_tensor(name, shape, dt,
                                          kind="ExternalOutput").ap()
        dout("d_ce", [C + 1, D], BF16)
        dout("d_semT", [128, DC * Q], FP8)
        dout("d_comb", [128, QC * D], FP8)
        dout("d_kpT", [128, DC * Q], FP8)
        dout("d_qpT", [128, DC * Q], FP8)
        dout("d_rowsum", [1, Q])
        dout("d_outacc", [128, DC * Q])
        dout("d_oT", [128, DC * Q], BF16)
        dout("d_normT", [128, DC * Q])

    # AllGather buffers, split so the kp gather can fire before comb is done.
    # Wave w = the w-th half of every rank's local keys. kp blocks are
    # kpT[:, w*512:+512] flattened ([1024 d, 512 k] row-major as [512, 1024]);
    # vv blocks are vv[w*512:+512, :] (natural [512 k, 1024 d]).
    cx.bounce_kp = [nc.dram_tensor(f"bkp{w}", [Q // 2, D], FP8,
                                   kind="Internal").ap() for w in range(2)]
    cx.bounce_vv = [nc.dram_tensor(f"bvv{w}", [Q // 2, D], FP8,
                                   kind="Internal").ap() for w in range(2)]
    cx.ag_kp = [nc.dram_tensor(f"agkp{w}", [NCORES * Q // 2, D], FP8,
                               kind="Internal", addr_space="Shared").ap()
                for w in range(2)]
    cx.ag_vv = [nc.dram_tensor(f"agvv{w}", [NCORES * Q // 2, D], FP8,
                               kind="Internal", addr_space="Shared").ap()
                for w in range(2)]

    with tile.TileContext(nc) as tc:
        with tc.tile_pool(name="pp", bufs=1) as pp:
            # all small fp32 constants packed into one 4KB-padded tile
            consts = pp.tile([128, 288], F32)
            cx.ident = consts[:, 0:128]
            make_identity(nc, cx.ident)
            cx.ones_c = consts[:, 128:129]        # ones col (partition reduce)
            nc.vector.memset(cx.ones_c, 1.0)
            cx.eps_t = consts[0:1, 129:130]
            nc.vector.memset(cx.eps_t, 1e-5)
            cx.ones_r = consts[0:1, 130:258]      # ones row (bias mm / bcast)
            nc.vector.memset(cx.ones_r, 1.0)
            cb = pp.tile([128, 130], BF16)
            cx.ones_cb = cb[:, 0:1]
            nc.vector.memset(cx.ones_cb, 1.0)
            cx.ones_rb = cb[0:1, 2:130]
            nc.vector.memset(cx.ones_rb, 1.0)
            # fp8 ones with stride-16 pair layout for DoubleRow rowsum MMs
            c8 = pp.tile([128, 2, 16], FP8)
            nc.vector.memset(c8[:], 1.0)
            cx.ones_pair8 = c8[:, :, 0:1]         # [128, 2, 1], step 16

            for _rep in range(repeat):
                with tc.tile_pool(name="pq", bufs=1) as pq:
                    cx.pq = pq
                    _phase_a(nc, tc, cx)
                    if cx.upto == "a":
                        nc.gpsimd.dma_start(
                            out=cx.out_d.rearrange("(c p) d -> p c d", p=128),
                            in_=cx.qpT8[:])
                    else:
                        with tc.tile_pool(name="pbc", bufs=1) as pbc:
                            cx.pbc = pbc
                            _phase_b(nc, tc, cx)
                            if cx.upto == "ab":
                                nc.sync.dma_start(
                                    out=cx.out_d
                                    .rearrange("(c p) d -> p c d", p=128),
                                    in_=cx.outn[:])
                            else:
                                _phase_c(nc, tc, cx)
    nc.compile()
    return nc


def _phase_a(nc, tc, cx):
    """Projections. Order chosen so attention unblocks ASAP: ce -> sem ->
    kp (+kp AllGather per wave) -> vis -> qp -> comb (+vv AllGather).
    All attention-path GEMMs are fp8 DoubleRow.
    """
    debug, dbg = cx.debug, cx.dbg

    def ship(bounce, agb, bounce_ap, in_ap):
        nc.sync.dma_start(out=bounce_ap, in_=in_ap)
        if cx.single:
            for r in range(NCORES):
                nc.sync.dma_start(
                    out=agb[r * (Q // 2):(r + 1) * (Q // 2), :], in_=bounce)
        else:
            nc.gpsimd.collective_compute(
                "AllGather", ALU.bypass,
                replica_groups=[list(range(NCORES))],
                ins=[bounce], outs=[agb])

    with (
        tc.tile_pool(name="paV", bufs=1) as paV,
        tc.tile_pool(name="pap", bufs=5, space="PSUM") as pap,
    ):
        oh_sb = paV.tile([C + 1, Q], BF16)
        nc.sync.dma_start(out=oh_sb[:], in_=cx.ohT)

        with tc.tile_pool(name="pa0", bufs=1) as pa0:
            # ce = [cemb;bg;1] @ [w_proj.T;b_proj]  -> [81, D]
            cembT_sb = pa0.tile([128, 3, C + 1], F32)
            nc.sync.dma_start(out=cembT_sb[:],
                              in_=cx.cembT.rearrange("(c p) n -> p c n", p=128))
            wprojT_sb = pa0.tile([128, 3, D], F32)
            nc.sync.dma_start(out=wprojT_sb[:],
                              in_=cx.wprojT.rearrange("(c p) d -> p c d", p=128))
            ce_ps = pap.tile([C + 1, D], F32, tag="ps2", bufs=1)
            for ob in range(2):
                for sc in range(3):
                    nc.tensor.matmul(ce_ps[:, ob * 512:(ob + 1) * 512],
                                     cembT_sb[:, sc, :],
                                     wprojT_sb[:, sc, ob * 512:(ob + 1) * 512],
                                     start=(sc == 0), stop=(sc == 2))
            ce_sb = pa0.tile([C + 1, D], BF16)
            nc.scalar.copy(out=ce_sb[:], in_=ce_ps[:])
            if debug:
                nc.sync.dma_start(out=dbg["d_ce"], in_=ce_sb[:])

            # sem gather (one pass): semT8 = ce[gt].T, semrT8 = relu(semT8)
            semT8 = pa0.tile([128, DC, Q], FP8)
            semrT8 = pa0.tile([128, DC, Q], FP8)
            for dc in range(DC):
                for qh in range(Q // 512):
                    qs = slice(qh * 512, (qh + 1) * 512)
                    sem_ps = pap.tile([128, 512], F32, tag="ps")
                    nc.tensor.matmul(sem_ps[:],
                                     ce_sb[:, dc * 128:(dc + 1) * 128],
                                     oh_sb[:, qs], start=True, stop=True)
                    nc.scalar.copy(out=semT8[:, dc, qs], in_=sem_ps[:])
                    nc.scalar.activation(out=semrT8[:, dc, qs], in_=sem_ps[:],
                                         func=AF.Relu)
            if debug:
                nc.sync.dma_start(out=dbg["d_semT"],
                                  in_=semT8[:].rearrange("p c q -> p (c q)"))

            fpT_sb = pa0.tile([128, DC, Q], BF16)
            nc.sync.dma_start(out=fpT_sb[:],
                              in_=cx.fpT.rearrange("(c p) q -> p c q", p=128))
            fpT8_sb = pa0.tile([128, DC, Q], FP8)
            nc.sync.dma_start(out=fpT8_sb[:],
                              in_=cx.fpT8.rearrange("(c p) q -> p c q", p=128))

            with tc.tile_pool(name="paK", bufs=1) as paK:
                wqk_sb = paK.tile([128, DC, D], FP8)
                nc.sync.dma_start(out=wqk_sb[:],
                                  in_=cx.wqkT8.rearrange("(c p) o -> p c o", p=128))

                # kp = wqk @ semrT, fp8 DoubleRow, shipped per wave ASAP
                kp_sb = paK.tile([128, DC, Q], FP8)
                for w in range(2):
                    qs = slice(w * 512, (w + 1) * 512)
                    for oc in range(DC):
                        qk_ps = pap.tile([128, 512], F32, tag="ps")
                        for ic in range(0, DC, 2):
                            nc.tensor.matmul(
                                qk_ps[:],
                                wqk_sb[:, ic:ic + 2, oc * 128:(oc + 1) * 128],
                                semrT8[:, ic:ic + 2, qs],
                                start=(ic == 0), stop=(ic == DC - 2),
                                perf_mode=DR)
                        nc.scalar.copy(out=kp_sb[:, oc, qs], in_=qk_ps[:])
                    ship(cx.bounce_kp[w], cx.ag_kp[w],
                         _mk_ap(cx.bounce_kp[w], 0,
                                [[512, 128], [65536, 8], [1, 512]]),
                         kp_sb[:, :, qs])
                if debug:
                    nc.sync.dma_start(out=dbg["d_kpT"],
                                      in_=kp_sb[:].rearrange("p c q -> p (c q)"))

                # visT = relu(fpT) bf16 (lives until phase C), fp8 copy for qp
                cx.visT = visT = cx.pq.tile([128, DC, Q], BF16, name="visT")
                visT8 = paK.tile([128, DC, Q], FP8)
                for dc in range(DC):
                    nc.scalar.activation(out=visT[:, dc, :], in_=fpT_sb[:, dc, :],
                                         func=AF.Relu)
                    nc.scalar.activation(out=visT8[:, dc, :], in_=fpT8_sb[:, dc, :],
                                         func=AF.Relu)

                # qp = wqk @ visT (queries unblock attention S matmuls)
                cx.qpT8 = cx.pq.tile([128, DC, Q], FP8, name="qpT8")
                for w in range(2):
                    qs = slice(w * 512, (w + 1) * 512)
                    for oc in range(DC):
                        qk_ps = pap.tile([128, 512], F32, tag="ps")
                        for ic in range(0, DC, 2):
                            nc.tensor.matmul(
                                qk_ps[:],
                                wqk_sb[:, ic:ic + 2, oc * 128:(oc + 1) * 128],
                                visT8[:, ic:ic + 2, qs],
                                start=(ic == 0), stop=(ic == DC - 2),
                                perf_mode=DR)
                        nc.scalar.copy(out=cx.qpT8[:, oc, qs], in_=qk_ps[:])
                if debug:
                    nc.sync.dma_start(out=dbg["d_qpT"],
                                      in_=cx.qpT8[:].rearrange("p c q -> p (c q)"))

            # comb (natural [q, d]) fp8 DoubleRow, shipped per wave.
            # bias b_comb added via DVE broadcast add (bcast row precomputed).
            comb_sb = paV.tile([128, QC, D], FP8, name="comb_sb")
            wcb_row = pa0.tile([1, D], F32)
            nc.sync.dma_start(out=wcb_row[:], in_=cx.wcb)
            bias_bc = pa0.tile([128, D], F32)
            for ob in range(2):
                os_ = slice(ob * 512, (ob + 1) * 512)
                bb_ps = pap.tile([128, 512], F32, tag="ps")
                nc.tensor.matmul(bb_ps[:], cx.ones_r, wcb_row[:, os_],
                                 start=True, stop=True)
                nc.scalar.copy(out=bias_bc[:, os_], in_=bb_ps[:])
            with tc.tile_pool(name="paw", bufs=2) as paw:
                for w in range(2):
                    for ob in range(2):
                        os_ = slice(ob * 512, (ob + 1) * 512)
                        wcq = paw.tile([128, 16, 512], FP8, tag="wcq")
                        nc.sync.dma_start(
                            out=wcq[:],
                            in_=cx.wcombT8[:, os_]
                            .rearrange("(c p) o -> p c o", p=128))
                        for qc in range(w * 4, w * 4 + 4):
                            cb_ps = pap.tile([128, 512], F32, tag="ps")
                            for ic in range(0, 16, 2):
                                lhs = (semT8[:, ic:ic + 2, qc * 128:(qc + 1) * 128]
                                       if ic < 8 else
                                       fpT8_sb[:, ic - 8:ic - 6,
                                               qc * 128:(qc + 1) * 128])
                                nc.tensor.matmul(cb_ps[:], lhs,
                                                 wcq[:, ic:ic + 2, :],
                                                 start=(ic == 0),
                                                 stop=(ic == 14),
                                                 perf_mode=DR)
                            nc.vector.tensor_add(comb_sb[:, qc, os_], cb_ps[:],
                                                 bias_bc[:, os_])
                    ship(cx.bounce_vv[w], cx.ag_vv[w],
                         _mk_ap(cx.bounce_vv[w], 0,
                                [[1024, 128], [131072, 4], [1, 1024]]),
                         comb_sb[:, w * 4:(w + 1) * 4, :])
            if debug:
                nc.sync.dma_start(out=dbg["d_comb"],
                                  in_=comb_sb[:].rearrange("p c q -> p (c q)"))


def _phase_b(nc, tc, cx):
    """Attention: S^T = kp^T-chunks x qpT, E = exp(S/32), out^T += vv^T E.
    All matmuls fp8 DoubleRow (256-contraction per instruction).

    N=512 structure: per 2048-key superblock j, first compute all 32 E tiles
    (16 key-chunks x 2 query-halves, free dim 512) into one [128,16,512]
    fp8 tile per query half, then do the PV matmuls in two d-half passes per
    query-half so the PV accumulator fits in 4 PSUM banks.
    PSUM: S 2 + rowsum 2 + PV 4 = 8 banks.
    """
    debug, dbg = cx.debug, cx.dbg
    qpT8 = cx.qpT8
    cx.rowsum = rowsum = cx.pbc.tile([1, Q], F32, name="rowsum")
    out_acc = cx.pbc.tile([128, DC, Q], F32, name="out_acc")
    with (
        tc.tile_pool(name="pb", bufs=1) as pb,
        tc.tile_pool(name="pkv", bufs=12) as pkv,
        tc.tile_pool(name="pe", bufs=4) as pe,
        tc.tile_pool(name="pbo", bufs=1, space="PSUM") as pbo,
        tc.tile_pool(name="pbs", bufs=2, space="PSUM") as pbs,
        tc.tile_pool(name="pbr", bufs=2, space="PSUM") as pbr,
    ):
        # dummy-key contribution to the softmax denominator
        dum_sb = pb.tile([128, DC, 16], FP8)
        nc.sync.dma_start(out=dum_sb[:],
                          in_=cx.dumT16.rearrange("(c p) o -> p c o", p=128))
        for qh in range(Q // 512):
            qs = slice(qh * 512, (qh + 1) * 512)
            sd_ps = pbs.tile([1, 512], F32, tag="sps")
            for dc in range(0, DC, 2):
                nc.tensor.matmul(sd_ps[:], dum_sb[:, dc:dc + 2, 0:1],
                                 qpT8[:, dc:dc + 2, qs],
                                 start=(dc == 0), stop=(dc == DC - 2),
                                 perf_mode=DR)
            nc.scalar.activation(out=rowsum[:, qs], in_=sd_ps[:],
                                 func=AF.Exp, scale=float(SCALE))

        for j in range(4):                # key superblocks of 2048 keys
            w, rr = j // 2, (j % 2) * 4
            kp_t, vv_t = [], []
            for s in range(4):
                r = rr + s
                kt = pkv.tile([128, DC, 512], FP8, tag="kp")
                nc.sync.dma_start(
                    out=kt[:],
                    in_=_mk_ap(cx.ag_kp[w], r * (Q // 2) * D,
                               [[512, 128], [65536, 8], [1, 512]]))
                vt = pkv.tile([128, 4, D], FP8, tag="vv")
                nc.sync.dma_start(
                    out=vt[:],
                    in_=cx.ag_vv[w][r * (Q // 2):(r + 1) * (Q // 2), :]
                    .rearrange("(kc p) d -> p kc d", p=128))
                kp_t.append(kt)
                vv_t.append(vt)

            # S + exp for all 16 key chunks x 2 query halves; accumulate
            # rowsum per sk pair (DoubleRow over the E tile).
            e_t = [pe.tile([128, 16, 512], FP8, tag="et", name=f"et{j}_{_qh}")
                   for _qh in range(2)]
            r_ps = [pbr.tile([1, 512], F32, tag="rps", name=f"rps{_qh}")
                    for _qh in range(2)]
            for sk in range(16):
                s, kc = sk // 4, sk % 4
                for qh in range(2):
                    qs = slice(qh * 512, (qh + 1) * 512)
                    s_ps = pbs.tile([128, 512], F32, tag="sps")
                    for dc in range(0, DC, 2):
                        nc.tensor.matmul(
                            s_ps[:],
                            kp_t[s][:, dc:dc + 2, kc * 128:(kc + 1) * 128],
                            qpT8[:, dc:dc + 2, qs],
                            start=(dc == 0), stop=(dc == DC - 2),
                            perf_mode=DR)
                    nc.scalar.activation(out=e_t[qh][:, sk, :], in_=s_ps[:],
                                         func=AF.Exp, scale=float(SCALE))
                    if sk % 2 == 1:
                        nc.tensor.matmul(r_ps[qh][:], cx.ones_pair8,
                                         e_t[qh][:, sk - 1:sk + 1, :],
                                         start=(sk == 1), stop=(sk == 15),
                                         perf_mode=DR)
            for qh in range(2):
                qs = slice(qh * 512, (qh + 1) * 512)
                nc.vector.tensor_add(rowsum[:, qs], rowsum[:, qs], r_ps[qh][:])

            # PV: two d-half passes per query half (4 PSUM banks each)
            for qh in range(2):
                qs = slice(qh * 512, (qh + 1) * 512)
                for dh in range(2):
                    o_ps = pbo.tile([128, 4, 512], F32, tag="ops")
                    for sk in range(0, 16, 2):
                        s, kc = sk // 4, sk % 4
                        for d4 in range(4):
                            dc = dh * 4 + d4
                            nc.tensor.matmul(
                                o_ps[:, d4, :],
                                vv_t[s][:, kc:kc + 2, dc * 128:(dc + 1) * 128],
                                e_t[qh][:, sk:sk + 2, :],
                                start=(sk == 0), stop=(sk == 14),
                                perf_mode=DR)
                    for d4 in range(4):
                        dc = dh * 4 + d4
                        if j == 0:
                            nc.vector.tensor_copy(out_acc[:, dc, qs],
                                                  o_ps[:, d4, :])
                        else:
                            nc.vector.tensor_add(out_acc[:, dc, qs],
                                                 out_acc[:, dc, qs],
                                                 o_ps[:, d4, :])

        if debug:
            nc.sync.dma_start(out=dbg["d_rowsum"], in_=rowsum[:])
            nc.sync.dma_start(out=dbg["d_outacc"],
                              in_=out_acc[:].rearrange("p c q -> p (c q)"))

        # normalize in place: out_acc /= rowsum (broadcast along partitions)
        recip = pb.tile([1, Q], F32)
        nc.vector.reciprocal(recip[:], rowsum[:])
        recipb = pb.tile([128, Q], F32)
        for qh in range(Q // 512):
            qs = slice(qh * 512, (qh + 1) * 512)
            b_ps = pbs.tile([128, 512], F32, tag="sps")
            nc.tensor.matmul(b_ps[:], cx.ones_r, recip[:, qs],
                             start=True, stop=True)
            nc.scalar.copy(out=recipb[:, qs], in_=b_ps[:])
        for dc in range(DC):
            nc.vector.tensor_mul(out_acc[:, dc, :], out_acc[:, dc, :], recipb[:])
        cx.outn = out_acc


def _phase_c(nc, tc, cx):
    """Epilogue: o1/o2/o3, LayerNorm, FFN, final relu-add, transpose, store."""
    debug, dbg = cx.debug, cx.dbg
    outn = cx.outn
    with (
        tc.tile_pool(name="pcB", bufs=1) as pcB,
        tc.tile_pool(name="pcp", bufs=8, space="PSUM") as pcp,
    ):
        # all per-feature bias vectors packed into one 4KB tile
        bias = pcB.tile([128, 48], F32)
        b1_sb = bias[:, 0:4]
        nc.sync.dma_start(out=b1_sb, in_=cx.b1.rearrange("(c p) -> p c", p=128))
        b2_sb = bias[:, 4:8]
        nc.sync.dma_start(out=b2_sb, in_=cx.b2.rearrange("(c p) -> p c", p=128))
        b3_sb = bias[:, 8:16]
        nc.sync.dma_start(out=b3_sb, in_=cx.b3.rearrange("(c p) -> p c", p=128))
        bf1_sb = bias[:, 16:24]
        nc.sync.dma_start(out=bf1_sb, in_=cx.bf1.rearrange("(c p) -> p c", p=128))
        bf2_sb = bias[:, 24:32]
        nc.sync.dma_start(out=bf2_sb, in_=cx.bf2.rearrange("(c p) -> p c", p=128))
        lnb2_sb = bias[:, 32:40]                 # ln_b + bf2 folded
        nc.sync.dma_start(out=lnb2_sb, in_=cx.lnb.rearrange("(c p) -> p c", p=128))
        nc.vector.tensor_add(lnb2_sb, lnb2_sb, bf2_sb)
        lng_sb = bias[:, 40:48]
        nc.sync.dma_start(out=lng_sb, in_=cx.lng.rearrange("(c p) -> p c", p=128))

        with tc.tile_pool(name="pcOT", bufs=1) as pcOT:
            oT_sb = pcOT.tile([128, DC, Q], BF16)
            oT32 = pcOT.tile([128, DC, Q], F32)   # fp32 copy for the LN path
            cx._oT32 = oT32

            with tc.tile_pool(name="pcA", bufs=1) as pcA:
                vis2 = cx.visT
                o1_sb = pcA.tile([128, 4, Q], BF16)
                o2_sb = pcA.tile([128, 4, Q], BF16)
                with tc.tile_pool(name="pcZ", bufs=1) as pcZ:
                    w1_sb = pcZ.tile([128, DC, 512], BF16)
                    nc.sync.dma_start(out=w1_sb[:],
                                      in_=cx.w1T.rearrange("(c p) o -> p c o", p=128))
                    w2_sb = pcZ.tile([128, DC, 512], BF16)
                    nc.sync.dma_start(out=w2_sb[:],
                                      in_=cx.w2T.rearrange("(c p) o -> p c o", p=128))
                    for half, (o_sb, wh_sb, bh_sb) in enumerate(
                            [(o1_sb, w1_sb, b1_sb), (o2_sb, w2_sb, b2_sb)]):
                        for qh in range(Q // 512):
                            qs = slice(qh * 512, (qh + 1) * 512)
                            z_sb = pcZ.tile([128, DC, 512], BF16, tag="z", bufs=1)
                            for dc in range(DC):
                                if half == 0:
                                    nc.vector.tensor_mul(z_sb[:, dc, :],
                                                         outn[:, dc, qs],
                                                         vis2[:, dc, qs])
                                else:
                                    nc.vector.tensor_sub(z_sb[:, dc, :],
                                                         vis2[:, dc, qs],
                                                         outn[:, dc, qs])
                            for oc in range(4):
                                m_ps = pcp.tile([128, 512], F32, tag="cps")
                                for ic in range(DC):
                                    nc.tensor.matmul(
                                        m_ps[:],
                                        wh_sb[:, ic, oc * 128:(oc + 1) * 128],
                                        z_sb[:, ic, :],
                                        start=(ic == 0), stop=(ic == DC - 1))
                                nc.scalar.activation(out=o_sb[:, oc, qs],
                                                     in_=m_ps[:], func=AF.Relu,
                                                     bias=bh_sb[:, oc:oc + 1])

                # o = w3 @ [o1; o2; vis] + b3  (transposed out [d, q])
                with tc.tile_pool(name="pcW", bufs=2) as pcW:
                    for oc in range(DC):
                        w3c = pcW.tile([128, 16, 128], BF16, tag="w3c")
                        nc.sync.dma_start(
                            out=w3c[:],
                            in_=cx.w3T[:, oc * 128:(oc + 1) * 128]
                            .rearrange("(c p) o -> p c o", p=128))
                        for qh in range(Q // 512):
                            qs = slice(qh * 512, (qh + 1) * 512)
                            m_ps = pcp.tile([128, 512], F32, tag="cps")
                            for ic in range(16):
                                rhs = (o1_sb[:, ic, qs] if ic < 4 else
                                       o2_sb[:, ic - 4, qs] if ic < 8 else
                                       vis2[:, ic - 8, qs])
                                nc.tensor.matmul(m_ps[:], w3c[:, ic, :], rhs,
                                                 start=(ic == 0), stop=(ic == 15))
                            nc.scalar.activation(out=oT_sb[:, oc, qs], in_=m_ps[:],
                                                 func=AF.Identity,
                                                 bias=b3_sb[:, oc:oc + 1])
                            nc.scalar.activation(out=oT32[:, oc, qs], in_=m_ps[:],
                                                 func=AF.Identity,
                                                 bias=b3_sb[:, oc:oc + 1])
            if debug:
                nc.sync.dma_start(out=dbg["d_oT"],
                                  in_=oT_sb[:].rearrange("p c q -> p (c q)"))

            with tc.tile_pool(name="pcN", bufs=1) as pcN:
                # LayerNorm over feature dim (partition reduce via ones-matmul)
                normT = pcN.tile([128, DC, Q], F32)
                with tc.tile_pool(name="pcL", bufs=2) as pcL:
                    for qh in range(Q // 512):
                        qs = slice(qh * 512, (qh + 1) * 512)
                        sum_ps = pcp.tile([1, 512], F32, tag="cps")
                        ssq_ps = pcp.tile([1, 512], F32, tag="cps")
                        for dc in range(DC):
                            nc.tensor.matmul(sum_ps[:], cx.ones_cb,
                                             oT_sb[:, dc, qs],
                                             start=(dc == 0), stop=(dc == DC - 1))
                            sq_t = pcL.tile([128, 512], BF16, tag="sq")
                            nc.scalar.activation(out=sq_t[:], in_=oT_sb[:, dc, qs],
                                                 func=AF.Square)
                            nc.tensor.matmul(ssq_ps[:], cx.ones_cb, sq_t[:],
                                             start=(dc == 0), stop=(dc == DC - 1))
                        st = pcL.tile([1, 3, 512], F32, tag="st", bufs=1)
                        slot_a, slot_b, slot_c = (st[:, i, :] for i in range(3))
                        nc.scalar.mul(out=slot_a, in_=sum_ps[:], mul=1.0 / D)  # mu
                        nc.scalar.mul(out=slot_b, in_=ssq_ps[:], mul=1.0 / D)  # E[x^2]
                        nc.vector.tensor_mul(slot_c, slot_a, slot_a)    # mu^2
                        nc.vector.tensor_sub(slot_b, slot_b, slot_c)    # var
                        nc.scalar.activation(out=slot_b, in_=slot_b, func=AF.Sqrt,
                                             bias=cx.eps_t)             # sd
                        nc.vector.reciprocal(slot_c, slot_b)            # c1 = rstd
                        nc.vector.tensor_mul(slot_a, slot_a, slot_c)    # c0 = mu*rstd
                        c1b = pcL.tile([128, 512], F32, tag="c1b")
                        c0b = pcL.tile([128, 512], F32, tag="c0b")
                        for src, dst in [(slot_c, c1b), (slot_a, c0b)]:
                            bb_ps = pcp.tile([128, 512], F32, tag="cps")
                            nc.tensor.matmul(bb_ps[:], cx.ones_r, src,
                                             start=True, stop=True)
                            nc.scalar.copy(out=dst[:], in_=bb_ps[:])
                        for dc in range(DC):
                            tmp = pcL.tile([128, 512], F32, tag="lnt")
                            nc.vector.tensor_mul(tmp[:], oT32[:, dc, qs], c1b[:])
                            nc.vector.tensor_sub(tmp[:], tmp[:], c0b[:])
                            nc.vector.tensor_scalar(
                                out=normT[:, dc, qs], in0=tmp[:],
                                scalar1=lng_sb[:, dc:dc + 1],
                                scalar2=lnb2_sb[:, dc:dc + 1],
                                op0=ALU.mult, op1=ALU.add)
                if debug:
                    nc.sync.dma_start(out=dbg["d_normT"],
                                      in_=normT[:].rearrange("p c q -> p (c q)"))

                # FFN layer 1 (consumes oT), weights streamed per output chunk
                f1_sb = pcN.tile([128, DC, Q], BF16)
                with tc.tile_pool(name="pcM1", bufs=2) as pcM1:
                    for oc in range(DC):
                        wf1c = pcM1.tile([128, DC, 128], BF16, tag="wf1c")
                        nc.sync.dma_start(
                            out=wf1c[:],
                            in_=cx.wf1T[:, oc * 128:(oc + 1) * 128]
                            .rearrange("(c p) o -> p c o", p=128))
                        for qh in range(Q // 512):
                            qs = slice(qh * 512, (qh + 1) * 512)
                            m_ps = pcp.tile([128, 512], F32, tag="cps")
                            for ic in range(DC):
                                nc.tensor.matmul(m_ps[:], wf1c[:, ic, :],
                                                 oT_sb[:, ic, qs],
                                                 start=(ic == 0),
                                                 stop=(ic == DC - 1))
                            nc.scalar.activation(out=f1_sb[:, oc, qs], in_=m_ps[:],
                                                 func=AF.Relu,
                                                 bias=bf1_sb[:, oc:oc + 1])

                # FFN layer 2 + LayerNorm residual + final relu
                with tc.tile_pool(name="pcM2", bufs=1) as pcM2:
                    fin_sb = pcM2.tile([128, DC, Q], F32)
                    with tc.tile_pool(name="pcM2w", bufs=2) as pcM2w:
                        for oc in range(DC):
                            wf2c = pcM2w.tile([128, DC, 128], BF16, tag="wf2c")
                            nc.sync.dma_start(
                                out=wf2c[:],
                                in_=cx.wf2T[:, oc * 128:(oc + 1) * 128]
                                .rearrange("(c p) o -> p c o", p=128))
                            for qh in range(Q // 512):
                                qs = slice(qh * 512, (qh + 1) * 512)
                                m_ps = pcp.tile([128, 512], F32, tag="cps")
                                for ic in range(DC):
                                    nc.tensor.matmul(m_ps[:], wf2c[:, ic, :],
                                                     f1_sb[:, ic, qs],
                                                     start=(ic == 0),
                                                     stop=(ic == DC - 1))
                                ts = pcM2w.tile([128, 512], F32, tag="ts")
                                nc.vector.tensor_add(ts[:], m_ps[:],
                                                     normT[:, oc, qs])
                                nc.scalar.activation(out=fin_sb[:, oc, qs],
                                                     in_=ts[:], func=AF.Relu)

                    # transpose [d, q] -> [q, d] on the PE
                    # (oT32 is dead after the LN stage; reuse its space)
                    onat = cx._oT32
                    for dc in range(DC):
                        for qc in range(QC):
                            t_ps = pcp.tile([128, 128], F32, tag="cps")
                            nc.tensor.transpose(
                                t_ps[:], fin_sb[:, dc, qc * 128:(qc + 1) * 128],
                                cx.ident)
                            nc.scalar.copy(
                                out=onat[:, qc, dc * 128:(dc + 1) * 128],
                                in_=t_ps[:])
                    nc.sync.dma_start(
                        out=cx.out_d.rearrange("(c p) d -> p c d", p=128),
                        in_=onat[:])


# ---------------------------------------------------------------------------
# Host side
# ---------------------------------------------------------------------------

_CACHE = {}
E4 = ml_dtypes.float8_e4m3


def _to8(x):
    return np.clip(np.asarray(x, np.float32), -240, 240).astype(E4)


def _prep_in_maps(inputs):
    f32 = np.float32
    fp = np.asarray(inputs["feature_pooled"], f32)
    gt = np.asarray(inputs["gt_classes"]).astype(np.int64)
    ce = np.asarray(inputs["class_embed"], f32)
    bg = np.asarray(inputs["bg_embed"], f32)
    w_proj = np.asarray(inputs["w_proj"], f32)
    b_proj = np.asarray(inputs["b_proj"], f32)
    w_comb = np.asarray(inputs["w_comb"], f32)
    b_comb = np.asarray(inputs["b_comb"], f32)
    w_qk = np.asarray(inputs["w_qk"], f32)
    dummy = np.asarray(inputs["dummy"], f32)

    cembT = np.zeros((384, C + 1), f32)
    cembT[:S] = np.concatenate([ce, bg], 0).T
    cembT[S] = 1.0
    wprojT = np.zeros((384, D), f32)
    wprojT[:S] = w_proj.T
    wprojT[S] = b_proj
    dumT16 = np.zeros((D, 16), f32)
    dumT16[:, 0] = dummy[0]

    shared = {
        "cembT": cembT,
        "wprojT": wprojT,
        "wcombT8": _to8(np.ascontiguousarray(w_comb.T)),
        "wcb": b_comb[None, :].astype(f32),
        "wqkT8": _to8(np.ascontiguousarray(w_qk.T)),
        "dumT16": _to8(dumT16),
        "w1T": np.ascontiguousarray(np.asarray(inputs["w1"], f32).T).astype(ml_dtypes.bfloat16),
        "b1": np.asarray(inputs["b1"], f32),
        "w2T": np.ascontiguousarray(np.asarray(inputs["w2"], f32).T).astype(ml_dtypes.bfloat16),
        "b2": np.asarray(inputs["b2"], f32),
        "w3T": np.ascontiguousarray(np.asarray(inputs["w3"], f32).T).astype(ml_dtypes.bfloat16),
        "b3": np.asarray(inputs["b3"], f32),
        "wf1T": np.ascontiguousarray(np.asarray(inputs["wf1"], f32).T).astype(ml_dtypes.bfloat16),
        "bf1": np.asarray(inputs["bf1"], f32),
        "wf2T": np.ascontiguousarray(np.asarray(inputs["wf2"], f32).T).astype(ml_dtypes.bfloat16),
        "bf2": np.asarray(inputs["bf2"], f32),
        "lng": np.asarray(inputs["ln_g"], f32),
        "lnb": np.asarray(inputs["ln_b"], f32),
    }
    in_maps = []
    for c in range(NCORES):
        qs = slice(c * Q, (c + 1) * Q)
        oh = np.zeros((C + 1, Q), ml_dtypes.bfloat16)
        oh[gt[qs], np.arange(Q)] = 1.0
        m = dict(shared)
        fpTc = np.ascontiguousarray(fp[qs].T)
        m["fpT"] = fpTc.astype(ml_dtypes.bfloat16)
        m["fpT8"] = _to8(fpTc)
        m["ohT"] = oh
        in_maps.append(m)
    return in_maps


def get_nc(debug=False):
    key = ("nc", debug)
    if key not in _CACHE:
        _CACHE[key] = build(debug=debug)
    return _CACHE[key]


def kernel(**inputs):
    from concourse import bass_utils
    try:
        # persistent XLA/PJRT compile cache so repeat invocations (fresh
        # processes included) skip the NEFF compile
        import jax
        jax.config.update("jax_compilation_cache_dir", "/tmp/jax_neff_cache")
        jax.config.update("jax_persistent_cache_min_compile_time_secs", 1.0)
        jax.config.update("jax_persistent_cache_min_entry_size_bytes", 0)
    except Exception:
        pass
    nc = get_nc(debug=False)
    in_maps = _prep_in_maps(inputs)
    res = bass_utils.run_bass_kernel_spmd(
        nc, in_maps, core_ids=list(range(NCORES)), trace=False)
    return np.concatenate([res.results[c]["out"] for c in range(NCORES)], axis=0)
